# revision 5
# baseline (speedup 1.0000x reference)
"""Self-contained TRN2 Bass kernel for the 16-head MHA problem.

kernel(**inputs) takes FULL inputs (x [4,2048,1024], Wq/Wk/Wv [16,1024,64],
Wo [1024,1024], bo [1024]) and returns the FULL output [4,2048,1024] f32.

Sharding over 8 NeuronCores: core c handles batch b = c//2 and head group
g = c%2 (8 of 16 heads) — tensor parallel over heads with the output
projection's input dim sharded; the 2-way partial-sum reduce per batch and
the bias add happen host-side on the gathered results.
"""
import sys

for _p in ("/opt/trn_rl_repo",):
    if _p not in sys.path:
        sys.path.insert(0, _p)

import numpy as np
import concourse.bass as bass
import concourse.mybir as mybir
from concourse import bacc
from concourse import library_config
from concourse.bass import ts, ds
from concourse.tile import TileContext
from concourse.vector_clock import ScopedClock
from concourse import bass_utils

F32 = mybir.dt.float32
BF16 = mybir.dt.bfloat16
AF = mybir.ActivationFunctionType

NUM_HEADS = 16
EMB = 1024
HEAD = 64
SEQ = 2048
BATCH = 4
N_CORES = 8


class TC(TileContext):
    """TileContext whose final drain splits its sem waits across SP NOPs —
    the CTRL instruction encoding holds only one wait and this env's Tile
    puts the whole global clock on the tail drain."""

    def _drain_and_barrier(self, tick_clock, wait_clock):
        nc = self.nc
        dummy = nc.sync.nop(nofuse=True)
        wait_clock.add_sem_waits(dummy.ins, ScopedClock({None: tick_clock.global_clock}))
        si = dummy.ins.sync_info
        waits = list(si.on_wait) if si is not None else []
        if len(waits) > 1:
            si.on_wait = waits[:1]
            sem_by_name = {h.name: h for h in self.sems.allocated().values()}
            for w in waits[1:]:
                nop = nc.sync.nop(nofuse=True)
                nop._wait_ge(sem_by_name[w.ant_name], w.wait_value)
        nc.sync.drain()
        nc.all_engine_barrier()
        popped = nc._tile_sem_poison_stack.pop()
        assert popped is self._sem_poison
        nc.clear_and_free_semaphores(list(self.sems.allocated().values()))
        nc.all_engine_barrier()


def build_mha_nc(S=SEQ, E=EMB, D=HEAD, H=NUM_HEADS // 2):
    """Single-core SPMD program; H = heads per core (pair-packed).

    Fully transposed formulation (see v1 notes): scoresT = kT.T @ qT with
    the two heads of a pair on PE row tiles (0,0)/(64,0) running
    concurrently; attnT = exp(scoresT) (one ACT + one causal affine_select
    per chunk covering both heads); ctxT+denominator = [1|v].T @ attnT
    accumulated over k chunks (ones column FIRST so the denominator lands
    on PSUM partition 0 and feeds reciprocal_approx_fast directly);
    out = concatT.T @ WoT accumulated over head-dim chunks, written as
    bf16 partials (host sums the pair + bias in f32).

    v2 schedule: j-outer group order (all 4 pairs at query-row j, then the
    row's output projection), input DMA in priority waves on the single
    HWDGE FIFO (x wave 0 + wv + pair-0 wq/wk first so attention starts
    ~15us earlier), and all projection/output work chopped into ~2-matmul
    "pieces" pumped between each chunk's score and ctx matmuls so the PE
    fills the exp-latency window instead of idling.
    """
    P = 128
    EC = E // P          # 8
    NQ = 512
    J = S // NQ          # 4  (query rows == DMA waves)
    KK = S // P          # 16
    NP = H // 2          # 4 pairs
    HD = H * D           # 512
    HC = HD // P         # 4
    NE = min(512, E)
    JE = E // NE         # 2

    nc = bacc.Bacc("TRN2", target_bir_lowering=False, debug=False)
    # host pre-casts x to bf16 in wave-major layout [P, J, EC, NQ] so each
    # wave is one dma_start with 8KB-contiguous per-partition runs
    xT_in = nc.dram_tensor("xT_in", [P, J, EC, NQ], BF16, kind="ExternalInput")
    wq_in = nc.dram_tensor("wq_in", [P, NP, EC, 2 * D], BF16, kind="ExternalInput")
    wk_in = nc.dram_tensor("wk_in", [P, NP, EC, 2 * D], BF16, kind="ExternalInput")
    wv_in = nc.dram_tensor("wv_in", [P, EC, H, D], BF16, kind="ExternalInput")
    wo_in = nc.dram_tensor("wo_in", [P, HC, E], BF16, kind="ExternalInput")
    out_p = nc.dram_tensor("out_p", [S, E], BF16, kind="ExternalOutput")

    with TC(nc) as tc:
        with (
            tc.tile_pool(name="const", bufs=1) as cpool,
            tc.tile_pool(name="persist", bufs=1) as pers,
            tc.tile_pool(name="stage", bufs=3) as stg,
            tc.tile_pool(name="attn", bufs=6) as apool,
            tc.tile_pool(name="small", bufs=3) as spool,
            tc.tile_pool(name="psS", bufs=2, space="PSUM") as psS,
            tc.tile_pool(name="psF", bufs=2, space="PSUM") as psF,
            tc.tile_pool(name="psC", bufs=1, space="PSUM") as psC,
        ):
            # memsets go on the Vector engine: the gpsimd LIBRARY_RELOAD for
            # partition_broadcast stalls the gpsimd queue ~10us, and the
            # warmup matmuls must not wait on it
            warm_w = cpool.tile([P, P], mybir.dt.float16, tag="warmw")
            nc.vector.memset(warm_w[:], 0.0)
            warm_x = cpool.tile([P, NQ], mybir.dt.float16, tag="warmx")
            nc.vector.memset(warm_x[:], 0.0)
            nc.gpsimd.load_library(library_config.attn)

            xT = pers.tile([P, J, EC, NQ], BF16, tag="xT")
            qT = pers.tile([P, NP, S], BF16, tag="qT")
            kT = pers.tile([P, NP, S], BF16, tag="kT")
            v_pad = pers.tile([P, KK, H, D + 1], BF16, tag="vp")
            woT = pers.tile([P, HC, E], BF16, tag="woT")
            concatT = pers.tile([P, NP, S], BF16, tag="concT")
            wq_bf = pers.tile([P, NP, EC, 2 * D], BF16, tag="wq")
            wk_bf = pers.tile([P, NP, EC, 2 * D], BF16, tag="wk")
            wv_bf = pers.tile([P, EC, H, D], BF16, tag="wv")

            # ones column LAST: denominator accumulates on PSUM partition 64
            # (multi-partition APs must start at partition 0/32/64/96, so the
            # ctx rows must sit at partitions 0-63)
            nc.vector.memset(v_pad[:, :, :, D:D + 1], 1.0)

            # HAM warmup: keep the PE busy on throwaway matmuls while the
            # first DMA wave streams, so real matmuls start at full clock.
            # full-K/M warms reach full HAM activity and unthrottle the PE.
            warm_ps = psS.tile([P, 2, NQ], F32, tag="spair", name="warm_ps")
            for _w in range(44):
                nc.tensor.matmul(warm_ps[:, 0, :], warm_w[:], warm_x[:],
                                 start=True, stop=True)

            # ---- input DMA: priority waves on the single qSPDynamicHW
            # FIFO. Wave 0 carries exactly what the first attention group
            # needs; wo comes last (first consumer is ~row 1). ----
            nc.sync.dma_start(xT[:, 0], xT_in[:, 0])
            nc.sync.dma_start(wv_bf[:], wv_in[:])
            nc.sync.dma_start(wq_bf[:, 0], wq_in[:, 0])
            nc.sync.dma_start(wk_bf[:, 0], wk_in[:, 0])
            nc.sync.dma_start(xT[:, 1], xT_in[:, 1])
            nc.sync.dma_start(wq_bf[:, 1:NP], wq_in[:, 1:NP])
            nc.sync.dma_start(wk_bf[:, 1:NP], wk_in[:, 1:NP])
            nc.sync.dma_start(xT[:, 2], xT_in[:, 2])
            nc.sync.dma_start(xT[:, 3], xT_in[:, 3])
            nc.sync.dma_start(woT[:], wo_in[:])

            # ---- filler machinery: chains of PE work yielded in ~2-matmul
            # pieces, pumped between attention chunks. Each chain owns one
            # 1-bank psF accumulator; chains are consumed strictly FIFO so
            # per-bank accumulation groups never interleave. ----
            fq = []

            def pump(n):
                cnt = 0
                while cnt < n and fq:
                    try:
                        next(fq[0])
                        cnt += 1
                    except StopIteration:
                        fq.pop(0)

            def chain_qk(p2, sc):
                # scale is folded into Wq host-side
                cnt = 0
                for w_sb, dst in ((wq_bf, qT), (wk_bf, kT)):
                    acc = psF.tile(
                        [P, NQ], F32, tag="facc",
                        name=f"qk_{p2}_{sc}_{0 if dst is qT else 1}",
                    )
                    for ec in range(EC):
                        nc.tensor.matmul(
                            acc[:], w_sb[:, p2, ec, :], xT[:, sc, ec, :],
                            start=(ec == 0), stop=(ec == EC - 1),
                        )
                        cnt += 1
                        if cnt % 2 == 0 and cnt < 2 * EC:
                            yield
                    nc.vector.tensor_copy(dst[:, p2, ts(sc, NQ)], acc[:])

            def chain_v(sc):
                acc = psF.tile([P, NQ], F32, tag="facc", name=f"vacc_{sc}")
                for ec in range(EC):
                    nc.tensor.matmul(
                        acc[:],
                        xT[:, sc // 4, ec, ds((sc % 4) * P, P)],
                        wv_bf[:, ec, :, :].rearrange("p h d -> p (h d)"),
                        start=(ec == 0), stop=(ec == EC - 1),
                    )
                    if ec % 2 == 1 and ec < EC - 1:
                        yield
                nc.vector.tensor_copy(
                    v_pad[:, sc, :, 0:D],
                    acc[:].rearrange("p (h d) -> p h d", d=D),
                )

            def chain_out(sc):
                # one full E-wide output row-block: 2 accumulations, one
                # [P, E] staging tile, one 2KB-per-partition DMA
                ot = stg.tile([P, E], BF16, tag="ostg", name=f"ot_{sc}")
                for n in range(JE):
                    acc = psF.tile([P, NE], F32, tag="facc",
                                   name=f"oacc_{sc}_{n}")
                    for hc in range(HC):
                        nc.tensor.matmul(
                            acc[:],
                            concatT[:, hc, ts(sc, P)],
                            woT[:, hc, ts(n, NE)],
                            start=(hc == 0), stop=(hc == HC - 1),
                        )
                        if hc % 2 == 1 and not (hc == HC - 1 and n == JE - 1):
                            yield
                    nc.vector.tensor_copy(ot[:, ts(n, NE)], acc[:])
                nc.sync.dma_start(out_p[ts(sc, P), :], ot[:])

            def emit_normalize(p2, j, ce):
                # ~51-ULP approx reciprocal of the PSUM denominator row.
                # custom-DVE ops mis-read non-zero base partitions, so the
                # PSUM den row (partition 64) is staged to partition 0 by a
                # plain copy before reciprocal_approx_fast
                dens = spool.tile([1, 2, NQ], F32, tag="dens",
                                  name=f"dens_{p2}_{j}")
                nc.vector.tensor_copy(dens[:], ce[ds(D, 1), :, :])
                recips = [
                    spool.tile([1, NQ], F32, tag=f"recip{hh}",
                               name=f"recip{hh}_{p2}_{j}")
                    for hh in range(2)
                ]
                for hh in range(2):
                    nc.vector.reciprocal_approx_fast(recips[hh][:],
                                                     dens[0:1, hh, :])
                for hh in range(2):
                    bc = spool.tile([D, NQ], F32, tag=f"bc{hh}",
                                    name=f"bc{hh}_{p2}_{j}")
                    nc.gpsimd.partition_broadcast(bc[:], recips[hh][:],
                                                  channels=D)
                    nc.vector.tensor_mul(
                        concatT[ds(hh * D, D), p2, ts(j, NQ)],
                        ce[0:D, hh, :], bc[:],
                    )

            pending = [None]

            def flush_pending():
                if pending[0] is not None:
                    p2x, jx, cex = pending[0]
                    emit_normalize(p2x, jx, cex)
                    pending[0] = None

            def emit_group(p2, j):
                # previous group's normalize first; the pump(8) right after
                # covers its DVE/gpsimd latency and the ce WAR with PE work
                flush_pending()
                pump(8)
                heads = (2 * p2, 2 * p2 + 1)
                n_kk = min(KK, 4 * j + 4)
                ce = psC.tile([D + 1, 2, NQ], F32, tag="ce",
                              name=f"ce_{p2}_{j}")
                for i in range(n_kk):
                    t = i - 4 * j  # >= 0 -> diagonal (partial) tile
                    q0 = P * t if t > 0 else 0
                    nq = NQ - q0
                    # both heads' score matmuls: row tiles (0,0)/(64,0), the
                    # PE executes them concurrently
                    sps = psS.tile([P, 2, NQ], F32, tag="spair",
                                   name=f"s2_{p2}_{j}_{i}")
                    for hh in range(2):
                        nc.tensor.matmul(
                            sps[:, hh, 0:nq],
                            kT[ds(hh * D, D), p2, ts(i, P)],
                            qT[ds(hh * D, D), p2, ds(j * NQ + q0, nq)],
                            start=True, stop=True,
                        )
                    at = apool.tile([P, 2, NQ], BF16, tag="at")
                    nc.scalar.activation(at[:, :, 0:nq], sps[:, :, 0:nq],
                                         AF.Exp)
                    if t >= 0:
                        # the invalid causal triangle (r > c) only touches
                        # the first 128 columns
                        nsel = min(P, nq)
                        nc.gpsimd.affine_select(
                            out=at[:, :, 0:nsel], in_=at[:, :, 0:nsel],
                            compare_op=mybir.AluOpType.is_ge,
                            fill=0.0, base=P * t - q0,
                            pattern=[[0, 2], [1, nsel]], channel_multiplier=-1,
                        )
                    # filler between scores and ctx: fills the exp window
                    pump(2)
                    for hh in range(2):
                        nc.tensor.matmul(
                            ce[0:D + 1, hh, ds(q0, nq)],
                            v_pad[:, i, heads[hh], :],
                            at[:, hh, 0:nq],
                            start=(i == 0), stop=(i == n_kk - 1),
                        )
                pending[0] = (p2, j, ce)

            # ---- early phase: v chunks 0-3 + pair-0 q/k (wave-0 data),
            # emitted whole so attention starts as soon as possible ----
            for sc in range(4):
                for _ in chain_v(sc):
                    pass
            for _ in chain_qk(0, 0):
                pass

            # row-0 fillers: remaining pairs' j=0 q/k, then wave-1 v chunks
            fq.append(chain_qk(1, 0))
            fq.append(chain_qk(2, 0))
            fq.append(chain_qk(3, 0))
            for sc in range(4, 8):
                fq.append(chain_v(sc))

            # ---- attention: j-outer so each query row's output projection
            # follows the row; fillers enqueued to arrive with their DMAs ----
            for j in range(J):
                for p2 in range(NP):
                    emit_group(p2, j)
                    if j + 1 < J:
                        fq.append(chain_qk(p2, j + 1))
                        vsc = 8 + 4 * j + p2  # v chunks 8..15 over rows 0-1
                        if vsc < KK:
                            fq.append(chain_v(vsc))
                # out(j) needs all 4 normalizes of row j; norm(3, j) is
                # flushed at the start of group(0, j+1), and fq is only
                # pumped after that flush, so enqueueing here is safe
                if j + 1 < J:
                    for sc in range(4 * j, 4 * j + 4):
                        fq.append(chain_out(sc))

            # tail: normalize the final group, then drain all remaining
            # fillers and the last row's output projection
            flush_pending()
            pump(1 << 30)
            for sc in range(4 * (J - 1), 4 * J):
                fq.append(chain_out(sc))
            pump(1 << 30)

    nc.finalize()
    return nc


_NC_CACHE = {}


def _get_nc():
    key = "mha"
    if key not in _NC_CACHE:
        _NC_CACHE[key] = build_mha_nc()
    return _NC_CACHE[key]


def _arr_xT(xb, bf16):
    # [S, E] f32 -> [P, J, EC, NQ] bf16 with xT[p, j, ec, q] = x[j*512+q, ec*128+p]
    P, NQ = 128, 512
    S, E = xb.shape
    v = xb.astype(bf16).reshape(S // NQ, NQ, E // P, P)
    return np.ascontiguousarray(v.transpose(3, 0, 2, 1))


def _arr_wqk(w, bf16):
    # [H, E, D] -> [P, NP, EC, 2*D] pair-packed lhsT layout
    H, E, D = w.shape
    P = 128
    v = w.astype(bf16).reshape(H // 2, 2, E // P, P, D)
    v = v.transpose(3, 0, 2, 1, 4)  # [P, NP, EC, 2, D]
    return np.ascontiguousarray(v.reshape(P, H // 2, E // P, 2 * D))


def _arr_wv(w, bf16):
    # [H, E, D] -> [P, EC, H, D]
    H, E, D = w.shape
    P = 128
    v = w.astype(bf16).reshape(H, E // P, P, D)
    return np.ascontiguousarray(v.transpose(2, 1, 0, 3))


def _arr_wo(w, bf16):
    # [E, HD] -> [P, HC, E] with woT[p, hc, e] = Wo[e, hc*128+p]
    E, HD = w.shape
    P = 128
    v = w.astype(bf16).T.reshape(HD // P, P, E)
    return np.ascontiguousarray(v.transpose(1, 0, 2))


def kernel(x, Wq, Wk, Wv, Wo, bo, _runner_kwargs=None):
    import ml_dtypes
    bf16 = ml_dtypes.bfloat16
    x = np.asarray(x, dtype=np.float32)
    Wq = np.asarray(Wq, dtype=np.float32)
    Wk = np.asarray(Wk, dtype=np.float32)
    Wv = np.asarray(Wv, dtype=np.float32)
    Wo = np.asarray(Wo, dtype=np.float32)
    bo = np.asarray(bo, dtype=np.float32)

    HPC = NUM_HEADS // 2  # heads per core
    HDS = HPC * HEAD      # concat-dim slice per core
    scale = HEAD ** -0.5

    nc = _get_nc()
    xbs = [_arr_xT(x[b], bf16) for b in range(BATCH)]
    wq_scaled = Wq * scale  # fold softmax scale into the q projection
    in_maps = []
    for c in range(N_CORES):
        b, g = c // 2, c % 2
        hs = slice(g * HPC, (g + 1) * HPC)
        in_maps.append({
            "xT_in": xbs[b],
            "wq_in": _arr_wqk(wq_scaled[hs], bf16),
            "wk_in": _arr_wqk(Wk[hs], bf16),
            "wv_in": _arr_wv(Wv[hs], bf16),
            "wo_in": _arr_wo(Wo[:, g * HDS:(g + 1) * HDS], bf16),
        })

    kw = dict(_runner_kwargs or {})
    res = bass_utils.run_bass_kernel_spmd(
        nc, in_maps, core_ids=list(range(N_CORES)), **kw
    )

    out = np.empty((BATCH, SEQ, EMB), dtype=np.float32)
    for b in range(BATCH):
        p0 = np.asarray(res.results[2 * b]["out_p"]).astype(np.float32)
        p1 = np.asarray(res.results[2 * b + 1]["out_p"]).astype(np.float32)
        out[b] = p0 + p1 + bo
    if kw.get("trace"):
        kernel.last_results = res
    return out


# revision 10
# speedup vs baseline: 1.0362x; 1.0362x over previous
"""Self-contained TRN2 Bass kernel for the 16-head MHA problem.

kernel(**inputs) takes FULL inputs (x [4,2048,1024], Wq/Wk/Wv [16,1024,64],
Wo [1024,1024], bo [1024]) and returns the FULL output [4,2048,1024] f32.

Sharding over 8 NeuronCores: core c handles batch b = c//2 and head group
g = c%2 (8 of 16 heads) — tensor parallel over heads with the output
projection's input dim sharded; the 2-way partial-sum reduce per batch and
the bias add happen host-side on the gathered results.
"""
import sys

for _p in ("/opt/trn_rl_repo",):
    if _p not in sys.path:
        sys.path.insert(0, _p)

import numpy as np
import concourse.bass as bass
import concourse.mybir as mybir
from concourse import bacc
from concourse import library_config
from concourse.bass import ts, ds
from concourse.tile import TileContext
from concourse.vector_clock import ScopedClock
from concourse import bass_utils

F32 = mybir.dt.float32
BF16 = mybir.dt.bfloat16
AF = mybir.ActivationFunctionType

NUM_HEADS = 16
EMB = 1024
HEAD = 64
SEQ = 2048
BATCH = 4
N_CORES = 8


class TC(TileContext):
    """TileContext whose final drain splits its sem waits across SP NOPs —
    the CTRL instruction encoding holds only one wait and this env's Tile
    puts the whole global clock on the tail drain."""

    def _drain_and_barrier(self, tick_clock, wait_clock):
        nc = self.nc
        dummy = nc.sync.nop(nofuse=True)
        wait_clock.add_sem_waits(dummy.ins, ScopedClock({None: tick_clock.global_clock}))
        si = dummy.ins.sync_info
        waits = list(si.on_wait) if si is not None else []
        if len(waits) > 1:
            si.on_wait = waits[:1]
            sem_by_name = {h.name: h for h in self.sems.allocated().values()}
            for w in waits[1:]:
                nop = nc.sync.nop(nofuse=True)
                nop._wait_ge(sem_by_name[w.ant_name], w.wait_value)
        nc.sync.drain()
        nc.all_engine_barrier()
        popped = nc._tile_sem_poison_stack.pop()
        assert popped is self._sem_poison
        nc.clear_and_free_semaphores(list(self.sems.allocated().values()))
        nc.all_engine_barrier()


def build_mha_nc(S=SEQ, E=EMB, D=HEAD, H=NUM_HEADS // 2):
    """Single-core SPMD program; H = heads per core (pair-packed).

    Fully transposed formulation (see v1 notes): scoresT = kT.T @ qT with
    the two heads of a pair on PE row tiles (0,0)/(64,0) running
    concurrently; attnT = exp(scoresT) (one ACT + one causal affine_select
    per chunk covering both heads); ctxT+denominator = [1|v].T @ attnT
    accumulated over k chunks (ones column FIRST so the denominator lands
    on PSUM partition 0 and feeds reciprocal_approx_fast directly);
    out = concatT.T @ WoT accumulated over head-dim chunks, written as
    bf16 partials (host sums the pair + bias in f32).

    v2 schedule: j-outer group order (all 4 pairs at query-row j, then the
    row's output projection), input DMA in priority waves on the single
    HWDGE FIFO (x wave 0 + wv + pair-0 wq/wk first so attention starts
    ~15us earlier), and all projection/output work chopped into ~2-matmul
    "pieces" pumped between each chunk's score and ctx matmuls so the PE
    fills the exp-latency window instead of idling.
    """
    P = 128
    EC = E // P          # 8
    NQ = 512
    J = S // NQ          # 4  (query rows == DMA waves)
    KK = S // P          # 16
    NP = H // 2          # 4 pairs
    HD = H * D           # 512
    HC = HD // P         # 4
    NE = min(512, E)
    JE = E // NE         # 2

    nc = bacc.Bacc("TRN2", target_bir_lowering=False, debug=False)
    # host pre-casts x to bf16 in wave-major layout [P, J, EC, NQ] so each
    # wave is one dma_start with 8KB-contiguous per-partition runs
    xT_in = nc.dram_tensor("xT_in", [P, J, EC, NQ], BF16, kind="ExternalInput")
    wq_in = nc.dram_tensor("wq_in", [P, NP, EC, 2 * D], BF16, kind="ExternalInput")
    wk_in = nc.dram_tensor("wk_in", [P, NP, EC, 2 * D], BF16, kind="ExternalInput")
    wv_in = nc.dram_tensor("wv_in", [P, EC, H, D], BF16, kind="ExternalInput")
    wo_in = nc.dram_tensor("wo_in", [P, HC, E], BF16, kind="ExternalInput")
    out_p = nc.dram_tensor("out_p", [S, E], BF16, kind="ExternalOutput")

    with TC(nc) as tc:
        with (
            tc.tile_pool(name="const", bufs=1) as cpool,
            tc.tile_pool(name="persist", bufs=1) as pers,
            tc.tile_pool(name="stage", bufs=3) as stg,
            tc.tile_pool(name="attn", bufs=6) as apool,
            tc.tile_pool(name="small", bufs=3) as spool,
            tc.tile_pool(name="psS", bufs=2, space="PSUM") as psS,
            tc.tile_pool(name="psF", bufs=2, space="PSUM") as psF,
            tc.tile_pool(name="psC", bufs=1, space="PSUM") as psC,
        ):
            # memsets go on the Vector engine: the gpsimd LIBRARY_RELOAD for
            # partition_broadcast stalls the gpsimd queue ~10us, and the
            # warmup matmuls must not wait on it
            warm_w = cpool.tile([P, P], mybir.dt.float16, tag="warmw")
            nc.vector.memset(warm_w[:], 0.0)
            warm_x = cpool.tile([P, NQ], mybir.dt.float16, tag="warmx")
            nc.vector.memset(warm_x[:], 0.0)
            nc.gpsimd.load_library(library_config.attn)

            xT = pers.tile([P, J, EC, NQ], BF16, tag="xT")
            qT = pers.tile([P, NP, S], BF16, tag="qT")
            kT = pers.tile([P, NP, S], BF16, tag="kT")
            v_pad = pers.tile([P, KK, H, D + 1], BF16, tag="vp")
            woT = pers.tile([P, HC, E], BF16, tag="woT")
            concatT = pers.tile([P, NP, S], BF16, tag="concT")
            wq_bf = pers.tile([P, NP, EC, 2 * D], BF16, tag="wq")
            wk_bf = pers.tile([P, NP, EC, 2 * D], BF16, tag="wk")
            wv_bf = pers.tile([P, EC, H, D], BF16, tag="wv")

            # ones column LAST: denominator accumulates on PSUM partition 64
            # (multi-partition APs must start at partition 0/32/64/96, so the
            # ctx rows must sit at partitions 0-63)
            nc.vector.memset(v_pad[:, :, :, D:D + 1], 1.0)

            # HAM warmup: keep the PE busy on throwaway matmuls while the
            # first DMA wave streams, so real matmuls start at full clock.
            # full-K/M warms reach full HAM activity and unthrottle the PE.
            warm_ps = psS.tile([P, 2, NQ], F32, tag="spair", name="warm_ps")
            for _w in range(44):
                nc.tensor.matmul(warm_ps[:, 0, :], warm_w[:], warm_x[:],
                                 start=True, stop=True)

            # ---- input DMA: priority waves on the single qSPDynamicHW
            # FIFO. Wave 0 carries exactly what the first attention group
            # needs; wo comes last (first consumer is ~row 1). ----
            nc.sync.dma_start(xT[:, 0], xT_in[:, 0])
            nc.sync.dma_start(wv_bf[:], wv_in[:])
            nc.sync.dma_start(wq_bf[:, 0], wq_in[:, 0])
            nc.sync.dma_start(wk_bf[:, 0], wk_in[:, 0])
            nc.sync.dma_start(xT[:, 1], xT_in[:, 1])
            nc.sync.dma_start(wq_bf[:, 1:NP], wq_in[:, 1:NP])
            nc.sync.dma_start(wk_bf[:, 1:NP], wk_in[:, 1:NP])
            nc.sync.dma_start(xT[:, 2], xT_in[:, 2])
            nc.sync.dma_start(xT[:, 3], xT_in[:, 3])
            nc.sync.dma_start(woT[:], wo_in[:])

            # ---- filler machinery: chains of PE work yielded in ~2-matmul
            # pieces, pumped between attention chunks. Each chain owns one
            # 1-bank psF accumulator; chains are consumed strictly FIFO so
            # per-bank accumulation groups never interleave. ----
            fq = []          # [key, generator] pairs, consumed strictly FIFO
            done = set()     # keys of fully-emitted chains

            def pump(n):
                cnt = 0
                while cnt < n and fq:
                    try:
                        next(fq[0][1])
                        cnt += 1
                    except StopIteration:
                        done.add(fq.pop(0)[0])

            def pump_until(keys):
                # force-drain the queue (FIFO) until every key has been
                # fully emitted — a group's scores/ctx must never precede
                # the emission of the q/k/v writes they read (Tile deps
                # are emission-order based)
                need = [k for k in keys if k not in done]
                while need:
                    assert fq, f"filler queue exhausted, still need {need}"
                    try:
                        next(fq[0][1])
                    except StopIteration:
                        done.add(fq.pop(0)[0])
                        need = [k for k in keys if k not in done]

            def chain_qk(p2, sc):
                # scale is folded into Wq host-side
                cnt = 0
                for w_sb, dst in ((wq_bf, qT), (wk_bf, kT)):
                    acc = psF.tile(
                        [P, NQ], F32, tag="facc",
                        name=f"qk_{p2}_{sc}_{0 if dst is qT else 1}",
                    )
                    for ec in range(EC):
                        nc.tensor.matmul(
                            acc[:], w_sb[:, p2, ec, :], xT[:, sc, ec, :],
                            start=(ec == 0), stop=(ec == EC - 1),
                        )
                        cnt += 1
                        if cnt % 2 == 0 and cnt < 2 * EC:
                            yield
                    nc.vector.tensor_copy(dst[:, p2, ts(sc, NQ)], acc[:])

            def chain_v(sc):
                acc = psF.tile([P, NQ], F32, tag="facc", name=f"vacc_{sc}")
                for ec in range(EC):
                    nc.tensor.matmul(
                        acc[:],
                        xT[:, sc // 4, ec, ds((sc % 4) * P, P)],
                        wv_bf[:, ec, :, :].rearrange("p h d -> p (h d)"),
                        start=(ec == 0), stop=(ec == EC - 1),
                    )
                    if ec % 2 == 1 and ec < EC - 1:
                        yield
                nc.vector.tensor_copy(
                    v_pad[:, sc, :, 0:D],
                    acc[:].rearrange("p (h d) -> p h d", d=D),
                )

            def chain_out(sc):
                # one full E-wide output row-block: 2 accumulations, one
                # [P, E] staging tile, one 2KB-per-partition DMA
                ot = stg.tile([P, E], BF16, tag="ostg", name=f"ot_{sc}")
                for n in range(JE):
                    acc = psF.tile([P, NE], F32, tag="facc",
                                   name=f"oacc_{sc}_{n}")
                    for hc in range(HC):
                        nc.tensor.matmul(
                            acc[:],
                            concatT[:, hc, ts(sc, P)],
                            woT[:, hc, ts(n, NE)],
                            start=(hc == 0), stop=(hc == HC - 1),
                        )
                        if hc % 2 == 1 and not (hc == HC - 1 and n == JE - 1):
                            yield
                    nc.vector.tensor_copy(ot[:, ts(n, NE)], acc[:])
                nc.sync.dma_start(out_p[ts(sc, P), :], ot[:])

            def emit_normalize(p2, j, ce):
                # ~51-ULP approx reciprocal of the PSUM denominator row.
                # custom-DVE ops mis-read non-zero base partitions, so the
                # PSUM den row (partition 64) is staged to partition 0 by a
                # plain copy before reciprocal_approx_fast
                dens = spool.tile([1, 2, NQ], F32, tag="dens",
                                  name=f"dens_{p2}_{j}")
                nc.vector.tensor_copy(dens[:], ce[ds(D, 1), :, :])
                recips = [
                    spool.tile([1, NQ], F32, tag=f"recip{hh}",
                               name=f"recip{hh}_{p2}_{j}")
                    for hh in range(2)
                ]
                for hh in range(2):
                    nc.vector.reciprocal_approx_fast(recips[hh][:],
                                                     dens[0:1, hh, :])
                for hh in range(2):
                    bc = spool.tile([D, NQ], F32, tag=f"bc{hh}",
                                    name=f"bc{hh}_{p2}_{j}")
                    nc.gpsimd.partition_broadcast(bc[:], recips[hh][:],
                                                  channels=D)
                    nc.vector.tensor_mul(
                        concatT[ds(hh * D, D), p2, ts(j, NQ)],
                        ce[0:D, hh, :], bc[:],
                    )

            pending = [None]

            def flush_pending():
                if pending[0] is not None:
                    p2x, jx, cex = pending[0]
                    emit_normalize(p2x, jx, cex)
                    pending[0] = None

            def emit_group(p2, j, rate=2, brate=8):
                # previous group's normalize first; the boundary pump right
                # after the first chunk's scores covers its DVE/gpsimd
                # latency and the ce WAR with PE work
                flush_pending()
                n_kk = min(KK, 4 * j + 4)
                pump_until([("qk", p2, jj) for jj in range(j + 1)]
                           + [("v", sc) for sc in range(n_kk)])
                heads = (2 * p2, 2 * p2 + 1)
                ce = psC.tile([D + 1, 2, NQ], F32, tag="ce",
                              name=f"ce_{p2}_{j}")
                for i in range(n_kk):
                    t = i - 4 * j  # >= 0 -> diagonal (partial) tile
                    q0 = P * t if t > 0 else 0
                    nq = NQ - q0
                    # both heads' score matmuls: row tiles (0,0)/(64,0), the
                    # PE executes them concurrently
                    sps = psS.tile([P, 2, NQ], F32, tag="spair",
                                   name=f"s2_{p2}_{j}_{i}")
                    for hh in range(2):
                        nc.tensor.matmul(
                            sps[:, hh, 0:nq],
                            kT[ds(hh * D, D), p2, ts(i, P)],
                            qT[ds(hh * D, D), p2, ds(j * NQ + q0, nq)],
                            start=True, stop=True,
                        )
                    at = apool.tile([P, 2, NQ], BF16, tag="at")
                    nc.scalar.activation(at[:, :, 0:nq], sps[:, :, 0:nq],
                                         AF.Exp)
                    if t >= 0:
                        # the invalid causal triangle (r > c) only touches
                        # the first 128 columns
                        nsel = min(P, nq)
                        nc.gpsimd.affine_select(
                            out=at[:, :, 0:nsel], in_=at[:, :, 0:nsel],
                            compare_op=mybir.AluOpType.is_ge,
                            fill=0.0, base=P * t - q0,
                            pattern=[[0, 2], [1, nsel]], channel_multiplier=-1,
                        )
                    # filler between scores and ctx: fills the exp window.
                    # The boundary pump comes after chunk 0's scores so the
                    # scalar's exp stream restarts before the filler burst.
                    pump(brate if i == 0 else rate)
                    for hh in range(2):
                        nc.tensor.matmul(
                            ce[0:D + 1, hh, ds(q0, nq)],
                            v_pad[:, i, heads[hh], :],
                            at[:, hh, 0:nq],
                            start=(i == 0), stop=(i == n_kk - 1),
                        )
                pending[0] = (p2, j, ce)

            # ---- early phase: v chunks 0-3 + pair-0 q/k (wave-0 data),
            # emitted whole so attention starts as soon as possible ----
            for sc in range(4):
                for _ in chain_v(sc):
                    pass
                done.add(("v", sc))
            for _ in chain_qk(0, 0):
                pass
            done.add(("qk", 0, 0))

            # row-0 fillers: remaining pairs' j=0 q/k, then wave-1 v chunks
            for p2 in (1, 2, 3):
                fq.append((("qk", p2, 0), chain_qk(p2, 0)))
            for sc in range(4, 8):
                fq.append((("v", sc), chain_v(sc)))

            # ---- attention: j-outer so each query row's output projection
            # follows the row; fillers enqueued to arrive with their DMAs ----
            # pump rates per row: rows 0-1 are PE-rich (small exp chunks,
            # dense q/k+v filler demand of their own) and must NOT vacuum
            # the filler queue; rows 2-3 are scalar-paced and need steady
            # filler or the HAM sees the PE idle and halves its clock
            RATES = ((1, 4), (1, 4), (2, 8), (2, 8))
            for j in range(J):
                rate, brate = RATES[j]
                for p2 in range(NP):
                    emit_group(p2, j, rate=rate, brate=brate)
                    if j + 1 < J:
                        fq.append((("qk", p2, j + 1), chain_qk(p2, j + 1)))
                        vsc = 8 + 4 * j + p2  # v chunks 8..15 over rows 0-1
                        if vsc < KK:
                            fq.append((("v", vsc), chain_v(vsc)))
                # out(j) needs all 4 normalizes of row j; norm(3, j) is
                # flushed at the start of group(0, j+1), and fq is only
                # pumped after that flush, so enqueueing at end of row j+1
                # is safe and keeps the out chains in reserve for the
                # filler-starved rows 2-3
                if j == 1:
                    for sc in range(0, 4):
                        fq.append((("out", sc), chain_out(sc)))
                elif j == 2:
                    for sc in range(4, 12):
                        fq.append((("out", sc), chain_out(sc)))

            # tail: normalize the final group, then drain all remaining
            # fillers and the last row's output projection
            flush_pending()
            pump(1 << 30)
            for sc in range(4 * (J - 1), 4 * J):
                fq.append((("out", sc), chain_out(sc)))
            pump(1 << 30)

    nc.finalize()
    return nc


_NC_CACHE = {}


def _get_nc():
    key = "mha"
    if key not in _NC_CACHE:
        _NC_CACHE[key] = build_mha_nc()
    return _NC_CACHE[key]


def _arr_xT(xb, bf16):
    # [S, E] f32 -> [P, J, EC, NQ] bf16 with xT[p, j, ec, q] = x[j*512+q, ec*128+p]
    P, NQ = 128, 512
    S, E = xb.shape
    v = xb.astype(bf16).reshape(S // NQ, NQ, E // P, P)
    return np.ascontiguousarray(v.transpose(3, 0, 2, 1))


def _arr_wqk(w, bf16):
    # [H, E, D] -> [P, NP, EC, 2*D] pair-packed lhsT layout
    H, E, D = w.shape
    P = 128
    v = w.astype(bf16).reshape(H // 2, 2, E // P, P, D)
    v = v.transpose(3, 0, 2, 1, 4)  # [P, NP, EC, 2, D]
    return np.ascontiguousarray(v.reshape(P, H // 2, E // P, 2 * D))


def _arr_wv(w, bf16):
    # [H, E, D] -> [P, EC, H, D]
    H, E, D = w.shape
    P = 128
    v = w.astype(bf16).reshape(H, E // P, P, D)
    return np.ascontiguousarray(v.transpose(2, 1, 0, 3))


def _arr_wo(w, bf16):
    # [E, HD] -> [P, HC, E] with woT[p, hc, e] = Wo[e, hc*128+p]
    E, HD = w.shape
    P = 128
    v = w.astype(bf16).T.reshape(HD // P, P, E)
    return np.ascontiguousarray(v.transpose(1, 0, 2))


def kernel(x, Wq, Wk, Wv, Wo, bo, _runner_kwargs=None):
    import ml_dtypes
    bf16 = ml_dtypes.bfloat16
    x = np.asarray(x, dtype=np.float32)
    Wq = np.asarray(Wq, dtype=np.float32)
    Wk = np.asarray(Wk, dtype=np.float32)
    Wv = np.asarray(Wv, dtype=np.float32)
    Wo = np.asarray(Wo, dtype=np.float32)
    bo = np.asarray(bo, dtype=np.float32)

    HPC = NUM_HEADS // 2  # heads per core
    HDS = HPC * HEAD      # concat-dim slice per core
    scale = HEAD ** -0.5

    nc = _get_nc()
    xbs = [_arr_xT(x[b], bf16) for b in range(BATCH)]
    wq_scaled = Wq * scale  # fold softmax scale into the q projection
    in_maps = []
    for c in range(N_CORES):
        b, g = c // 2, c % 2
        hs = slice(g * HPC, (g + 1) * HPC)
        in_maps.append({
            "xT_in": xbs[b],
            "wq_in": _arr_wqk(wq_scaled[hs], bf16),
            "wk_in": _arr_wqk(Wk[hs], bf16),
            "wv_in": _arr_wv(Wv[hs], bf16),
            "wo_in": _arr_wo(Wo[:, g * HDS:(g + 1) * HDS], bf16),
        })

    kw = dict(_runner_kwargs or {})
    res = bass_utils.run_bass_kernel_spmd(
        nc, in_maps, core_ids=list(range(N_CORES)), **kw
    )

    out = np.empty((BATCH, SEQ, EMB), dtype=np.float32)
    for b in range(BATCH):
        p0 = np.asarray(res.results[2 * b]["out_p"]).astype(np.float32)
        p1 = np.asarray(res.results[2 * b + 1]["out_p"]).astype(np.float32)
        out[b] = p0 + p1 + bo
    if kw.get("trace"):
        kernel.last_results = res
    return out


# revision 11
# speedup vs baseline: 1.1037x; 1.0652x over previous
"""Self-contained TRN2 Bass kernel for the 16-head MHA problem.

kernel(**inputs) takes FULL inputs (x [4,2048,1024], Wq/Wk/Wv [16,1024,64],
Wo [1024,1024], bo [1024]) and returns the FULL output [4,2048,1024] f32.

Sharding over 8 NeuronCores: core c handles batch b = c//2 and head group
g = c%2 (8 of 16 heads) — tensor parallel over heads with the output
projection's input dim sharded; the 2-way partial-sum reduce per batch and
the bias add happen host-side on the gathered results.
"""
import sys

for _p in ("/opt/trn_rl_repo",):
    if _p not in sys.path:
        sys.path.insert(0, _p)

import numpy as np
import concourse.bass as bass
import concourse.mybir as mybir
from concourse import bacc
from concourse import library_config
from concourse.bass import ts, ds
from concourse.tile import TileContext
from concourse.vector_clock import ScopedClock
from concourse import bass_utils

F32 = mybir.dt.float32
BF16 = mybir.dt.bfloat16
AF = mybir.ActivationFunctionType

NUM_HEADS = 16
EMB = 1024
HEAD = 64
SEQ = 2048
BATCH = 4
N_CORES = 8


class TC(TileContext):
    """TileContext whose final drain splits its sem waits across SP NOPs —
    the CTRL instruction encoding holds only one wait and this env's Tile
    puts the whole global clock on the tail drain."""

    def _drain_and_barrier(self, tick_clock, wait_clock):
        nc = self.nc
        dummy = nc.sync.nop(nofuse=True)
        wait_clock.add_sem_waits(dummy.ins, ScopedClock({None: tick_clock.global_clock}))
        si = dummy.ins.sync_info
        waits = list(si.on_wait) if si is not None else []
        if len(waits) > 1:
            si.on_wait = waits[:1]
            sem_by_name = {h.name: h for h in self.sems.allocated().values()}
            for w in waits[1:]:
                nop = nc.sync.nop(nofuse=True)
                nop._wait_ge(sem_by_name[w.ant_name], w.wait_value)
        nc.sync.drain()
        nc.all_engine_barrier()
        popped = nc._tile_sem_poison_stack.pop()
        assert popped is self._sem_poison
        nc.clear_and_free_semaphores(list(self.sems.allocated().values()))
        nc.all_engine_barrier()


def build_mha_nc(S=SEQ, E=EMB, D=HEAD, H=NUM_HEADS // 2):
    """Single-core SPMD program; H = heads per core (pair-packed).

    Fully transposed formulation (see v1 notes): scoresT = kT.T @ qT with
    the two heads of a pair on PE row tiles (0,0)/(64,0) running
    concurrently; attnT = exp(scoresT) (one ACT + one causal affine_select
    per chunk covering both heads); ctxT+denominator = [1|v].T @ attnT
    accumulated over k chunks (ones column FIRST so the denominator lands
    on PSUM partition 0 and feeds reciprocal_approx_fast directly);
    out = concatT.T @ WoT accumulated over head-dim chunks, written as
    bf16 partials (host sums the pair + bias in f32).

    v2 schedule: j-outer group order (all 4 pairs at query-row j, then the
    row's output projection), input DMA in priority waves on the single
    HWDGE FIFO (x wave 0 + wv + pair-0 wq/wk first so attention starts
    ~15us earlier), and all projection/output work chopped into ~2-matmul
    "pieces" pumped between each chunk's score and ctx matmuls so the PE
    fills the exp-latency window instead of idling.
    """
    P = 128
    EC = E // P          # 8
    NQ = 512
    J = S // NQ          # 4  (query rows == DMA waves)
    KK = S // P          # 16
    NP = H // 2          # 4 pairs
    HD = H * D           # 512
    HC = HD // P         # 4
    NE = min(512, E)
    JE = E // NE         # 2

    nc = bacc.Bacc("TRN2", target_bir_lowering=False, debug=False)
    # host pre-casts x to bf16 in wave-major layout [P, J, EC, NQ] so each
    # wave is one dma_start with 8KB-contiguous per-partition runs
    xT_in = nc.dram_tensor("xT_in", [P, J, EC, NQ], BF16, kind="ExternalInput")
    wq_in = nc.dram_tensor("wq_in", [P, NP, EC, 2 * D], BF16, kind="ExternalInput")
    wk_in = nc.dram_tensor("wk_in", [P, NP, EC, 2 * D], BF16, kind="ExternalInput")
    wv_in = nc.dram_tensor("wv_in", [P, EC, H, D], BF16, kind="ExternalInput")
    wo_in = nc.dram_tensor("wo_in", [P, HC, E], BF16, kind="ExternalInput")
    out_p = nc.dram_tensor("out_p", [S, E], BF16, kind="ExternalOutput")

    with TC(nc) as tc:
        with (
            tc.tile_pool(name="const", bufs=1) as cpool,
            tc.tile_pool(name="persist", bufs=1) as pers,
            tc.tile_pool(name="stage", bufs=3) as stg,
            tc.tile_pool(name="attn", bufs=6) as apool,
            tc.tile_pool(name="small", bufs=3) as spool,
            tc.tile_pool(name="psS", bufs=2, space="PSUM") as psS,
            tc.tile_pool(name="psF", bufs=2, space="PSUM") as psF,
            tc.tile_pool(name="psC", bufs=1, space="PSUM") as psC,
        ):
            # memsets go on the Vector engine: the gpsimd LIBRARY_RELOAD for
            # partition_broadcast stalls the gpsimd queue ~10us, and the
            # warmup matmuls must not wait on it
            warm_w = cpool.tile([P, P], mybir.dt.float16, tag="warmw")
            nc.vector.memset(warm_w[:], 0.0)
            warm_x = cpool.tile([P, NQ], mybir.dt.float16, tag="warmx")
            nc.vector.memset(warm_x[:], 0.0)
            nc.gpsimd.load_library(library_config.attn)

            xT = pers.tile([P, J, EC, NQ], BF16, tag="xT")
            qT = pers.tile([P, NP, S], BF16, tag="qT")
            kT = pers.tile([P, NP, S], BF16, tag="kT")
            v_pad = pers.tile([P, KK, H, D + 1], BF16, tag="vp")
            woT = pers.tile([P, HC, E], BF16, tag="woT")
            concatT = pers.tile([P, NP, S], BF16, tag="concT")
            wq_bf = pers.tile([P, NP, EC, 2 * D], BF16, tag="wq")
            wk_bf = pers.tile([P, NP, EC, 2 * D], BF16, tag="wk")
            wv_bf = pers.tile([P, EC, H, D], BF16, tag="wv")

            # ones column LAST: denominator accumulates on PSUM partition 64
            # (multi-partition APs must start at partition 0/32/64/96, so the
            # ctx rows must sit at partitions 0-63)
            nc.vector.memset(v_pad[:, :, :, D:D + 1], 1.0)

            # HAM warmup: keep the PE busy on throwaway matmuls while the
            # first DMA wave streams, so real matmuls start at full clock.
            # full-K/M warms reach full HAM activity and unthrottle the PE.
            warm_ps = psS.tile([P, 2, NQ], F32, tag="spair", name="warm_ps")
            for _w in range(44):
                nc.tensor.matmul(warm_ps[:, 0, :], warm_w[:], warm_x[:],
                                 start=True, stop=True)

            # ---- input DMA: priority waves on the single qSPDynamicHW
            # FIFO. Wave 0 carries exactly what the first attention group
            # needs; wo comes last (first consumer is ~row 1). ----
            nc.sync.dma_start(xT[:, 0], xT_in[:, 0])
            nc.sync.dma_start(wv_bf[:], wv_in[:])
            nc.sync.dma_start(wq_bf[:, 0], wq_in[:, 0])
            nc.sync.dma_start(wk_bf[:, 0], wk_in[:, 0])
            nc.sync.dma_start(xT[:, 1], xT_in[:, 1])
            nc.sync.dma_start(wq_bf[:, 1:NP], wq_in[:, 1:NP])
            nc.sync.dma_start(wk_bf[:, 1:NP], wk_in[:, 1:NP])
            nc.sync.dma_start(xT[:, 2], xT_in[:, 2])
            nc.sync.dma_start(xT[:, 3], xT_in[:, 3])
            nc.sync.dma_start(woT[:], wo_in[:])

            # ---- filler machinery: chains of PE work yielded in ~2-matmul
            # pieces, pumped between attention chunks. Each chain owns one
            # 1-bank psF accumulator; chains are consumed strictly FIFO so
            # per-bank accumulation groups never interleave. ----
            fq = []          # [key, generator] pairs, consumed strictly FIFO
            done = set()     # keys of fully-emitted chains

            def pump(n):
                cnt = 0
                while cnt < n and fq:
                    try:
                        next(fq[0][1])
                        cnt += 1
                    except StopIteration:
                        done.add(fq.pop(0)[0])

            def pump_until(keys):
                # force-drain the queue (FIFO) until every key has been
                # fully emitted — a group's scores/ctx must never precede
                # the emission of the q/k/v writes they read (Tile deps
                # are emission-order based)
                need = [k for k in keys if k not in done]
                while need:
                    assert fq, f"filler queue exhausted, still need {need}"
                    try:
                        next(fq[0][1])
                    except StopIteration:
                        done.add(fq.pop(0)[0])
                        need = [k for k in keys if k not in done]

            def chain_qk(p2, sc):
                # scale is folded into Wq host-side
                cnt = 0
                for w_sb, dst in ((wq_bf, qT), (wk_bf, kT)):
                    acc = psF.tile(
                        [P, NQ], F32, tag="facc",
                        name=f"qk_{p2}_{sc}_{0 if dst is qT else 1}",
                    )
                    for ec in range(EC):
                        nc.tensor.matmul(
                            acc[:], w_sb[:, p2, ec, :], xT[:, sc, ec, :],
                            start=(ec == 0), stop=(ec == EC - 1),
                        )
                        cnt += 1
                        if cnt < 2 * EC:
                            yield
                    nc.vector.tensor_copy(dst[:, p2, ts(sc, NQ)], acc[:])

            def chain_v(sc):
                acc = psF.tile([P, NQ], F32, tag="facc", name=f"vacc_{sc}")
                for ec in range(EC):
                    nc.tensor.matmul(
                        acc[:],
                        xT[:, sc // 4, ec, ds((sc % 4) * P, P)],
                        wv_bf[:, ec, :, :].rearrange("p h d -> p (h d)"),
                        start=(ec == 0), stop=(ec == EC - 1),
                    )
                    if ec < EC - 1:
                        yield
                nc.vector.tensor_copy(
                    v_pad[:, sc, :, 0:D],
                    acc[:].rearrange("p (h d) -> p h d", d=D),
                )

            def chain_out(sc):
                # one full E-wide output row-block: 2 accumulations, one
                # [P, E] staging tile, one 2KB-per-partition DMA
                ot = stg.tile([P, E], BF16, tag="ostg", name=f"ot_{sc}")
                for n in range(JE):
                    acc = psF.tile([P, NE], F32, tag="facc",
                                   name=f"oacc_{sc}_{n}")
                    for hc in range(HC):
                        nc.tensor.matmul(
                            acc[:],
                            concatT[:, hc, ts(sc, P)],
                            woT[:, hc, ts(n, NE)],
                            start=(hc == 0), stop=(hc == HC - 1),
                        )
                        if not (hc == HC - 1 and n == JE - 1):
                            yield
                    nc.vector.tensor_copy(ot[:, ts(n, NE)], acc[:])
                nc.sync.dma_start(out_p[ts(sc, P), :], ot[:])

            def emit_normalize(p2, j, ce):
                # ~51-ULP approx reciprocal of the PSUM denominator row.
                # custom-DVE ops mis-read non-zero base partitions, so the
                # PSUM den row (partition 64) is staged to partition 0 by a
                # plain copy before reciprocal_approx_fast
                dens = spool.tile([1, 2, NQ], F32, tag="dens",
                                  name=f"dens_{p2}_{j}")
                nc.vector.tensor_copy(dens[:], ce[ds(D, 1), :, :])
                recips = [
                    spool.tile([1, NQ], F32, tag=f"recip{hh}",
                               name=f"recip{hh}_{p2}_{j}")
                    for hh in range(2)
                ]
                for hh in range(2):
                    nc.vector.reciprocal_approx_fast(recips[hh][:],
                                                     dens[0:1, hh, :])
                for hh in range(2):
                    bc = spool.tile([D, NQ], F32, tag=f"bc{hh}",
                                    name=f"bc{hh}_{p2}_{j}")
                    nc.gpsimd.partition_broadcast(bc[:], recips[hh][:],
                                                  channels=D)
                    nc.vector.tensor_mul(
                        concatT[ds(hh * D, D), p2, ts(j, NQ)],
                        ce[0:D, hh, :], bc[:],
                    )

            pending = [None]

            def flush_pending():
                if pending[0] is not None:
                    p2x, jx, cex = pending[0]
                    emit_normalize(p2x, jx, cex)
                    pending[0] = None

            def emit_group(p2, j, rate=2, brate=8):
                # previous group's normalize first; the boundary pump right
                # after the first chunk's scores covers its DVE/gpsimd
                # latency and the ce WAR with PE work
                flush_pending()
                n_kk = min(KK, 4 * j + 4)
                pump_until([("qk", p2, jj) for jj in range(j + 1)]
                           + [("v", sc) for sc in range(n_kk)])
                heads = (2 * p2, 2 * p2 + 1)
                ce = psC.tile([D + 1, 2, NQ], F32, tag="ce",
                              name=f"ce_{p2}_{j}")
                for i in range(n_kk):
                    t = i - 4 * j  # >= 0 -> diagonal (partial) tile
                    q0 = P * t if t > 0 else 0
                    nq = NQ - q0
                    # both heads' score matmuls: row tiles (0,0)/(64,0), the
                    # PE executes them concurrently
                    sps = psS.tile([P, 2, NQ], F32, tag="spair",
                                   name=f"s2_{p2}_{j}_{i}")
                    for hh in range(2):
                        nc.tensor.matmul(
                            sps[:, hh, 0:nq],
                            kT[ds(hh * D, D), p2, ts(i, P)],
                            qT[ds(hh * D, D), p2, ds(j * NQ + q0, nq)],
                            start=True, stop=True,
                        )
                    at = apool.tile([P, 2, NQ], BF16, tag="at")
                    nc.scalar.activation(at[:, :, 0:nq], sps[:, :, 0:nq],
                                         AF.Exp)
                    if t >= 0:
                        # the invalid causal triangle (r > c) only touches
                        # the first 128 columns
                        nsel = min(P, nq)
                        nc.gpsimd.affine_select(
                            out=at[:, :, 0:nsel], in_=at[:, :, 0:nsel],
                            compare_op=mybir.AluOpType.is_ge,
                            fill=0.0, base=P * t - q0,
                            pattern=[[0, 2], [1, nsel]], channel_multiplier=-1,
                        )
                    # filler between scores and ctx: fills the exp window.
                    # The boundary pump comes after chunk 0's scores so the
                    # scalar's exp stream restarts before the filler burst.
                    pump(brate if i == 0 else rate)
                    for hh in range(2):
                        nc.tensor.matmul(
                            ce[0:D + 1, hh, ds(q0, nq)],
                            v_pad[:, i, heads[hh], :],
                            at[:, hh, 0:nq],
                            start=(i == 0), stop=(i == n_kk - 1),
                        )
                pending[0] = (p2, j, ce)

            # ---- early phase: v chunks 0-3 + pair-0 q/k (wave-0 data),
            # emitted whole so attention starts as soon as possible ----
            for sc in range(4):
                for _ in chain_v(sc):
                    pass
                done.add(("v", sc))
            for _ in chain_qk(0, 0):
                pass
            done.add(("qk", 0, 0))

            # row-0 fillers: remaining pairs' j=0 q/k, then wave-1 v chunks
            for p2 in (1, 2, 3):
                fq.append((("qk", p2, 0), chain_qk(p2, 0)))
            for sc in range(4, 8):
                fq.append((("v", sc), chain_v(sc)))

            # ---- attention: j-outer so each query row's output projection
            # follows the row; fillers enqueued to arrive with their DMAs ----
            # pump rates per row: rows 0-1 are PE-rich (small exp chunks,
            # dense q/k+v filler demand of their own) and must NOT vacuum
            # the filler queue; rows 2-3 are scalar-paced and need steady
            # filler or the HAM sees the PE idle and halves its clock
            RATES = ((2, 8), (2, 8), (1, 8), (1, 8))
            for j in range(J):
                rate, brate = RATES[j]
                for p2 in range(NP):
                    emit_group(p2, j, rate=rate, brate=brate)
                    if j + 1 < J:
                        fq.append((("qk", p2, j + 1), chain_qk(p2, j + 1)))
                        vsc = 8 + 4 * j + p2  # v chunks 8..15 over rows 0-1
                        if vsc < KK:
                            fq.append((("v", vsc), chain_v(vsc)))
                # out(j) needs all 4 normalizes of row j; norm(3, j) is
                # flushed at the start of group(0, j+1), and fq is only
                # pumped after that flush, so enqueueing at end of row j+1
                # is safe and keeps the out chains in reserve for the
                # filler-starved rows 2-3
                if j == 1:
                    for sc in range(0, 4):
                        fq.append((("out", sc), chain_out(sc)))
                elif j == 2:
                    for sc in range(4, 12):
                        fq.append((("out", sc), chain_out(sc)))

            # tail: normalize the final group, then drain all remaining
            # fillers and the last row's output projection
            flush_pending()
            pump(1 << 30)
            for sc in range(4 * (J - 1), 4 * J):
                fq.append((("out", sc), chain_out(sc)))
            pump(1 << 30)

    nc.finalize()
    return nc


_NC_CACHE = {}


def _get_nc():
    key = "mha"
    if key not in _NC_CACHE:
        _NC_CACHE[key] = build_mha_nc()
    return _NC_CACHE[key]


def _arr_xT(xb, bf16):
    # [S, E] f32 -> [P, J, EC, NQ] bf16 with xT[p, j, ec, q] = x[j*512+q, ec*128+p]
    P, NQ = 128, 512
    S, E = xb.shape
    v = xb.astype(bf16).reshape(S // NQ, NQ, E // P, P)
    return np.ascontiguousarray(v.transpose(3, 0, 2, 1))


def _arr_wqk(w, bf16):
    # [H, E, D] -> [P, NP, EC, 2*D] pair-packed lhsT layout
    H, E, D = w.shape
    P = 128
    v = w.astype(bf16).reshape(H // 2, 2, E // P, P, D)
    v = v.transpose(3, 0, 2, 1, 4)  # [P, NP, EC, 2, D]
    return np.ascontiguousarray(v.reshape(P, H // 2, E // P, 2 * D))


def _arr_wv(w, bf16):
    # [H, E, D] -> [P, EC, H, D]
    H, E, D = w.shape
    P = 128
    v = w.astype(bf16).reshape(H, E // P, P, D)
    return np.ascontiguousarray(v.transpose(2, 1, 0, 3))


def _arr_wo(w, bf16):
    # [E, HD] -> [P, HC, E] with woT[p, hc, e] = Wo[e, hc*128+p]
    E, HD = w.shape
    P = 128
    v = w.astype(bf16).T.reshape(HD // P, P, E)
    return np.ascontiguousarray(v.transpose(1, 0, 2))


def kernel(x, Wq, Wk, Wv, Wo, bo, _runner_kwargs=None):
    import ml_dtypes
    bf16 = ml_dtypes.bfloat16
    x = np.asarray(x, dtype=np.float32)
    Wq = np.asarray(Wq, dtype=np.float32)
    Wk = np.asarray(Wk, dtype=np.float32)
    Wv = np.asarray(Wv, dtype=np.float32)
    Wo = np.asarray(Wo, dtype=np.float32)
    bo = np.asarray(bo, dtype=np.float32)

    HPC = NUM_HEADS // 2  # heads per core
    HDS = HPC * HEAD      # concat-dim slice per core
    scale = HEAD ** -0.5

    nc = _get_nc()
    xbs = [_arr_xT(x[b], bf16) for b in range(BATCH)]
    wq_scaled = Wq * scale  # fold softmax scale into the q projection
    in_maps = []
    for c in range(N_CORES):
        b, g = c // 2, c % 2
        hs = slice(g * HPC, (g + 1) * HPC)
        in_maps.append({
            "xT_in": xbs[b],
            "wq_in": _arr_wqk(wq_scaled[hs], bf16),
            "wk_in": _arr_wqk(Wk[hs], bf16),
            "wv_in": _arr_wv(Wv[hs], bf16),
            "wo_in": _arr_wo(Wo[:, g * HDS:(g + 1) * HDS], bf16),
        })

    kw = dict(_runner_kwargs or {})
    res = bass_utils.run_bass_kernel_spmd(
        nc, in_maps, core_ids=list(range(N_CORES)), **kw
    )

    out = np.empty((BATCH, SEQ, EMB), dtype=np.float32)
    for b in range(BATCH):
        p0 = np.asarray(res.results[2 * b]["out_p"]).astype(np.float32)
        p1 = np.asarray(res.results[2 * b + 1]["out_p"]).astype(np.float32)
        out[b] = p0 + p1 + bo
    if kw.get("trace"):
        kernel.last_results = res
    return out
